# revision 12
# baseline (speedup 1.0000x reference)
"""Trainium2 Bass kernel for the vq_codebook problem.

  dist_sq[n,k] = sum_d (x[n,d]-ctrs[k,d])^2 * s[d]
  out = softmax(-dist_sq, axis=1) @ values

Sharding: data-parallel over N (8192 rows of x per core); ctrs/values/s
replicated on all 8 cores. No collectives (forward only).

Math: softmax is shift-invariant, so with cross = (x*s) @ ctrs.T and
c_sq[k] = sum_d s[d]*ctrs[k,d]^2,
  softmax(-dist_sq)[n,k] ∝ e^{2*cross[n,k] - c_sq[k]}.
The per-k factor e^{-c_sq[k]} commutes with the phase-2 contraction over
k, so it is folded into the codebook once at setup:
  vals''[k,:] = e^{C2 - c_sq[k]} * [values[k,:] | 1 | 1]
  E''[k,n]    = e^{2*cross[n,k] - C}
  y[n,:]      = (E''.T @ vals'')[n,:256] / (E''.T @ vals'')[n,256]
Global shifts C=12, C2=30 keep every stored intermediate inside
bf16/fp32 normal range (2*cross ∈ [-88,88] worst case over 8M normal
samples; c_sq ∈ ~[30,105]; row-max of 2*cross-c_sq ≥ -27, so
denominators ≥ e^{-27+C2-C} = e^{-9}); the shifts cancel in the ratio.

Layouts: phase 1 runs transposed (k on partitions, n on free) with
stationary lhs1[c] = s*ctrs^T chunk (contraction 64) so exp needs only
scalar scale/bias; x tiles are transposed on the PE. Phase 2 uses E
chunks as the stationary operand against vals'', producing y in natural
[n, d_out] layout. ~20 dummy matmuls at t=0 warm the PE's HAM clock
gate (1.2 -> 2.4 GHz) before real work arrives.
"""

import os

os.environ.setdefault("JAX_PLATFORMS", "axon")

import numpy as np

N, D_IN, K, D_OUT = 65536, 64, 1024, 256
NCORES = 8
NS = N // NCORES  # 8192 rows per core
TROWS = 512  # rows of x per tile
NTILES = NS // TROWS  # 16
KC = K // 128  # 8 centroid chunks
NSUB = TROWS // 128  # 4 output sub-tiles per tile
RAMP = 128  # rows per mini-tile during startup ramp (first TROWS rows)

USE_F32R = True
# Measured on the reference distribution: 2*cross in [-119, 156.1],
# rowmax(2*cross) >= 31.6, c_sq in [34.3, 110.8], rowmax(2*cross-c_sq)
# >= -26.4. C_E keeps E'' = e^{2cross-C_E} <= e^{86} < bf16 max e^{88.7}
# while the per-row top weight stays >= e^{-38.4} (bf16 normal); C_V
# keeps vals'' = e^{C_V-c_sq}*vals within bf16 normals, and the
# denominator >= e^{C_V-C_E+rowmax(arg)} = e^{-66.4} (fp32 normal).
C_E = 70.0  # E'' = exp(2*cross - C_E)
C_V = 30.0  # vals'' = exp(C_V - c_sq) * vals

_cache = {}


def _build(use_f32r, rows=NS):
    import concourse.bacc as bacc
    import concourse.tile as tile
    from concourse import masks, mybir

    f32 = mybir.dt.float32
    # Tiles feeding fp32r matmuls must be *written* as float32r (the engine
    # rounds on write; the BIR verifier enforces it), so the operand tiles
    # are allocated with the matmul dtype rather than bitcast at use.
    mmdt = mybir.dt.float32r if use_f32r else f32
    # Phase-2 operands in bf16: E is written by the exp activation and
    # vals by the setup-time scaling, so both get rounded on write; bf16
    # stationary weights get fast-weight-load on the PE.
    p2dt = mybir.dt.bfloat16
    Exp = mybir.ActivationFunctionType.Exp
    Copy = mybir.ActivationFunctionType.Copy

    ntiles = rows // TROWS
    nc = bacc.Bacc("TRN2", target_bir_lowering=False, debug=False)
    dma_start = nc.sync.dma_start
    x = nc.declare_dram_parameter("x", [rows, D_IN], f32, isOutput=False)
    ctrs = nc.declare_dram_parameter("ctrs", [K, D_IN], f32, isOutput=False)
    values = nc.declare_dram_parameter("values", [K, D_OUT], f32, isOutput=False)
    s = nc.declare_dram_parameter("s", [D_IN], f32, isOutput=False)
    y = nc.declare_dram_parameter("y", [rows, D_OUT], f32, isOutput=True)

    with tile.TileContext(nc) as tc:
        with (
            tc.tile_pool(name="const", bufs=1) as constp,
            tc.tile_pool(name="tmp1", bufs=2) as tmp1p,
            tc.tile_pool(name="xt", bufs=4) as xtp,
            tc.tile_pool(name="xsT", bufs=3) as xsTp,
            tc.tile_pool(name="E", bufs=3) as Ep,
            tc.tile_pool(name="ysb", bufs=3) as yp,
            tc.tile_pool(name="rcp", bufs=8) as rcpp,
            tc.tile_pool(name="psA", bufs=2, space="PSUM") as psA,
            tc.tile_pool(name="psX", bufs=2, space="PSUM") as psX,
            tc.tile_pool(name="psO", bufs=2, space="PSUM") as psO,
        ):
            ident = constp.tile([128, 128], f32)
            masks.make_identity(nc, ident[:])
            cv_col = constp.tile([128, 1], f32)
            nc.vector.memset(cv_col[:], C_V)
            ce_col = constp.tile([128, 1], f32)
            nc.vector.memset(ce_col[:], -C_E)

            # -------- startup-critical prefetch: tile-0 x DMA first --------
            def phase1_load(n0, trows):
                nsub = trows // 128
                xt = xtp.tile([128, nsub, D_IN], f32)
                dma_start(
                    xt[:], x[n0 : n0 + trows, :].rearrange("(a p) d -> p a d", p=128)
                )
                xsT = xsTp.tile([D_IN, trows], mmdt)
                for p in range((nsub + 1) // 2):
                    # Paired transpose: [128, 2, 64] -> [128, 128] PSUM with
                    # x_{2p}^T on partitions 0..63 and x_{2p+1}^T on 64..127.
                    pair = min(2, nsub - 2 * p)
                    xp = psX.tile([128, 128], f32, tag="psX")
                    nc.tensor.transpose(
                        xp[0 : 64 * pair, :],
                        xt[:, 2 * p : 2 * p + pair, :].rearrange("q a d -> q (a d)"),
                        ident[:],
                    )
                    c0 = 2 * p * 128
                    nc.vector.tensor_copy(xsT[:, c0 : c0 + 128], xp[0:64, :])
                    if pair == 2:
                        # Upper half shifts partitions 64..127 -> 0..63 via
                        # the engine write crossbar (shifted output base).
                        nc.vector.tensor_copy(
                            xsT[:, c0 + 128 : c0 + 256], xp[64:128, :]
                        )
                return xsT

            xsT0 = phase1_load(0, RAMP)

            # ---------------- constants ----------------
            s_col = constp.tile([D_IN, 1], f32)
            dma_start(s_col[:], s[:].rearrange("(p o) -> p o", o=1))
            ctrs_nat = constp.tile([128, KC, D_IN], f32)
            dma_start(
                ctrs_nat[:], ctrs[:].rearrange("(c p) d -> p c d", p=128)
            )

            # lhs1[0:64, c, :] = s[d] * ctrs^T chunk  (d on partitions)
            # csq_ps[:, c]     = sum_d s[d] * ctrs[k,d]^2  (k on partitions)
            lhs1 = constp.tile([D_IN, KC, 128], mmdt)
            csq_ps = psO.tile([128, KC], f32, tag="psO")
            for c in range(KC):
                tp = psX.tile([D_IN, TROWS], f32, tag="psX")
                nc.tensor.transpose(tp[:, 0:128], ctrs_nat[:, c, :], ident[:])
                nc.scalar.activation(
                    lhs1[:, c, :], tp[:, 0:128], Copy, scale=s_col[:]
                )
                tmp = tmp1p.tile([D_IN, 128], f32)
                nc.scalar.square(tmp[:], tp[:, 0:128])
                # csq[k] = sum_d ctrs^T[d,k]^2 * s[d] via a N=1 matmul
                nc.tensor.matmul(csq_ps[:, c : c + 1], tmp[:], s_col[:])
            fexp = constp.tile([128, KC], f32)
            nc.scalar.activation(fexp[:], csq_ps[:], Exp, bias=cv_col[:], scale=-1.0)

            # vals''[k, :] = e^{C_V - c_sq[k]} * [values | 1 | 1]
            vals_stage = constp.tile([128, KC, D_OUT + 2], f32)
            dma_start(
                vals_stage[:, :, 0:D_OUT],
                values[:].rearrange("(c p) v -> p c v", p=128),
            )
            nc.vector.memset(vals_stage[:, :, D_OUT : D_OUT + 2], 1.0)
            vals = constp.tile([128, KC, D_OUT + 2], p2dt)
            for c in range(KC):
                nc.vector.tensor_scalar_mul(
                    vals[:, c, :], vals_stage[:, c, :], fexp[:, c : c + 1]
                )

            # ---------------- main loop ----------------
            def phase1_mm(xsT, trows):
                E = Ep.tile([128, KC, trows], p2dt)
                for c in range(0, KC, 2):
                    pe = psA.tile([128, 2, trows], f32, tag="psA")
                    nc.tensor.matmul(pe[:, 0, :], lhs1[:, c, :], xsT[:])
                    nc.tensor.matmul(pe[:, 1, :], lhs1[:, c + 1, :], xsT[:])
                    nc.scalar.activation(
                        E[:, c : c + 2, :], pe[:], Exp, scale=2.0, bias=ce_col[:]
                    )
                return E

            def phase2(n0, trows, E):
                nsub = trows // 128
                ysb = yp.tile([128, nsub, D_OUT], f32)
                for a in range(nsub):
                    po = psO.tile([128, D_OUT + 2], f32, tag="psO")
                    for c in range(KC):
                        nc.tensor.matmul(
                            po[:],
                            E[:, c, a * 128 : (a + 1) * 128],
                            vals[:, c, :],
                            start=(c == 0),
                            stop=(c == KC - 1),
                        )
                    rcp = rcpp.tile([128, 1], f32)
                    nc.vector.reciprocal(rcp[:], po[:, D_OUT : D_OUT + 1])
                    nc.vector.tensor_scalar_mul(ysb[:, a, :], po[:, 0:D_OUT], rcp[:])
                dma_start(
                    y[n0 : n0 + trows, :].rearrange("(a p) v -> p a v", p=128),
                    ysb[:],
                )

            # Ramp: small tiles first so phase-2 work reaches the PE before
            # the exp-gated phase-1 pipeline fills (psA double-buffering
            # serializes chunk pairs behind the scalar engine early on).
            tiles = [(j * RAMP, RAMP) for j in range(TROWS // RAMP)]
            tiles += [(i * TROWS, TROWS) for i in range(1, ntiles)]
            prev = None
            for idx, (n0, trows) in enumerate(tiles):
                xsT = xsT0 if idx == 0 else phase1_load(n0, trows)
                Ecur = phase1_mm(xsT, trows)
                if prev is not None:
                    phase2(prev[0], prev[1], prev[2])
                prev = (n0, trows, Ecur)
            phase2(prev[0], prev[1], prev[2])

    nc.compile()
    nc.finalize()
    return nc


def get_nc(use_f32r=USE_F32R, rows=NS):
    key = ("nc", use_f32r, rows)
    if key not in _cache:
        _cache[key] = _build(use_f32r, rows)
    return _cache[key]


def make_in_maps(x, ctrs, values, s):
    x = np.ascontiguousarray(x, dtype=np.float32)
    ctrs = np.ascontiguousarray(ctrs, dtype=np.float32)
    values = np.ascontiguousarray(values, dtype=np.float32)
    s = np.ascontiguousarray(s, dtype=np.float32)
    return [
        {
            "x": x[i * NS : (i + 1) * NS],
            "ctrs": ctrs,
            "values": values,
            "s": s,
        }
        for i in range(NCORES)
    ]


def run(x, ctrs, values, s, trace=False, use_f32r=USE_F32R, tmpdir=None):
    from concourse.bass_utils import run_bass_kernel_spmd

    nc = get_nc(use_f32r)
    res = run_bass_kernel_spmd(
        nc,
        make_in_maps(x, ctrs, values, s),
        list(range(NCORES)),
        trace=trace,
        tmpdir=tmpdir,
    )
    out = np.concatenate([res.results[i]["y"] for i in range(NCORES)], axis=0)
    return out, res


def kernel(x, ctrs, values, s):
    out, _ = run(x, ctrs, values, s, trace=False)
    return out.astype(np.float32)
